# revision 41
# baseline (speedup 1.0000x reference)
"""Trainium2 Bass kernel for chess-structured multi-head attention (8 cores).

Math (per board b of 2048, S=64 squares, D=512, H=8 heads, HD=64):
  q/k/v = x @ W{q,k,v}.T + b{q,k,v}
  scores_h = q_h k_h^T / 8, masked per head (6 static chess relations,
  ray, attack), softmax over targets, out = concat_h(attn_h v_h) @ Wo.T + bo

Sharding: pure data parallel, 256 boards per core; weights replicated.

Per-core layout strategy (all matmul contractions need K on partitions):
  - x is fed pre-transposed from host: XT (512, 16384) bf16.
  - QT/KT computed transposed (d on partitions) directly: lhsT=WqT chunk,
    rhs=XT chunk.  V computed natural (tokens on partitions): lhsT=XT chunk
    (stationary), rhs=WvT chunk.
  - scores computed natural per (board b, head h): (64 s, 64 t) into a PSUM
    bank holding one board-pair (128, 512): partition half = board parity,
    64-col block = head.  16 small MMs share the bank via one start/stop
    accumulation group (disjoint writes).
  - masking is additive pre-scale (-240 -> -30 after *1/8): one static tile
    (heads 0-5) + per-board-pair dynamic tile (ray|attack), DVE adds.
  - exp on ScalarE (bf16 out), denominators via DVE sub-dim reduce, then
    normalize with a free-dim step-0 broadcast multiply.
  - attn transposed on the PE (128x128 identity transposes, bf16), which
    block-swaps board/head parity; consumed against V or a partition-swapped
    copy of V (cheap SBUF->SBUF DMA) so stationary/moving partition bases
    always match.
  - attn@v gives attnout TRANSPOSED (d on partitions) which feeds the output
    projection as the stationary operand: Y natural (tokens on partitions),
    written straight to DRAM.
"""

import os
import sys
from contextlib import ExitStack

import numpy as np
import ml_dtypes

for _p in ("/opt/trn_rl_repo", os.path.expanduser("~/.axon_site/_ro/trn_rl_repo")):
    if os.path.isdir(_p) and _p not in sys.path:
        sys.path.append(_p)

import concourse.bass as bass
import concourse.tile as tile
from concourse import bacc, mybir
from concourse.bass_utils import run_bass_kernel_spmd

BF16 = mybir.dt.bfloat16
F32 = mybir.dt.float32
bf16 = ml_dtypes.bfloat16

B, S, D, H, HD = 2048, 64, 512, 8, 64
NCORES = 8
BPC = B // NCORES          # boards per core
SCALE = float(1.0 / np.sqrt(HD))
MASKNEG = -240.0           # additive mask value pre-scale (-30 after *SCALE)


def _static_masks():
    sq = np.arange(64)
    r = sq // 8
    f = sq % 8
    ri, rj = r[:, None], r[None, :]
    fi, fj = f[:, None], f[None, :]
    dr = np.abs(ri - rj)
    df = np.abs(fi - fj)
    eye = np.eye(64, dtype=bool)
    file_m = fi == fj
    rank_m = ri == rj
    diag_m = (ri - fi) == (rj - fj)
    adiag_m = (ri + fi) == (rj + fj)
    knight_m = (((dr == 2) & (df == 1)) | ((dr == 1) & (df == 2))) | eye
    king_m = (dr <= 1) & (df <= 1)
    return np.stack([file_m, rank_m, diag_m, adiag_m, knight_m, king_m])


def build_nc(n_boards=BPC, stages=99):
    """Build the single-core Bass program (SPMD across 8 cores)."""
    assert n_boards % 8 == 0
    n_g8 = n_boards // 8          # groups of 8 boards (512 tokens)
    TOK = n_boards * S

    # Bacc (not plain Bass): its compile() pass splits multi-sem waits into
    # EventSemaphore instructions — walrus allows 1 wait per instruction.
    nc = bacc.Bacc(None)

    xt = nc.dram_tensor("xt", [4, 128, TOK], BF16, kind="ExternalInput")
    dynm = nc.dram_tensor("dynm", [n_boards // 2, 128, 128], BF16,
                          kind="ExternalInput")
    wqt = nc.dram_tensor("wqt", [4, 128, 512], BF16, kind="ExternalInput")
    wkt = nc.dram_tensor("wkt", [4, 128, 512], BF16, kind="ExternalInput")
    wvt = nc.dram_tensor("wvt", [4, 128, 512], BF16, kind="ExternalInput")
    wot = nc.dram_tensor("wot", [4, 128, 512], BF16, kind="ExternalInput")
    bqk = nc.dram_tensor("bqk", [128, 8], F32, kind="ExternalInput")
    bvb = nc.dram_tensor("bvb", [128, 512], F32, kind="ExternalInput")
    bob = nc.dram_tensor("bob", [128, 512], F32, kind="ExternalInput")
    statm = nc.dram_tensor("statm", [128, 384], BF16, kind="ExternalInput")
    identd = nc.dram_tensor("identd", [128, 128], BF16, kind="ExternalInput")
    out = nc.dram_tensor("out", [TOK, 512], F32, kind="ExternalOutput")

    AF = mybir.ActivationFunctionType
    ALU = mybir.AluOpType

    def chain(insts):
        # Order-only deps: the start=True MM must execute first and the
        # stop=True MM last; members in between may reorder freely.
        for b in insts[1:]:
            tile.add_dep_helper(b.ins, insts[0].ins, sync=False,
                                reason="psum group start-first")
        for a in insts[1:-1]:
            tile.add_dep_helper(insts[-1].ins, a.ins, sync=False,
                                reason="psum group stop-last")

    with tile.TileContext(nc) as tc, ExitStack() as ctx:
        const = ctx.enter_context(tc.tile_pool(name="const", bufs=1))
        # SBUF working pools
        p_xt = ctx.enter_context(tc.tile_pool(name="xt", bufs=3))
        p_qkv = ctx.enter_context(tc.tile_pool(name="qkv", bufs=3))
        p_att = ctx.enter_context(tc.tile_pool(name="att", bufs=3))
        p_sc = ctx.enter_context(tc.tile_pool(name="sc", bufs=3))
        p_out = ctx.enter_context(tc.tile_pool(name="out", bufs=3))
        # PSUM pools (8 banks total; every tile here is exactly one bank)
        ps_proj = ctx.enter_context(
            tc.tile_pool(name="ps_proj", bufs=3, space="PSUM"))
        ps_sc = ctx.enter_context(
            tc.tile_pool(name="ps_sc", bufs=2, space="PSUM"))
        ps_at = ctx.enter_context(
            tc.tile_pool(name="ps_at", bufs=1, space="PSUM"))
        ps_ao = ctx.enter_context(
            tc.tile_pool(name="ps_ao", bufs=2, space="PSUM"))

        # ---- constants ----
        w_sb = {}
        for name, t in (("wqt", wqt), ("wkt", wkt), ("wvt", wvt), ("wot", wot)):
            w = const.tile([128, 4, 512], BF16, tag=name)
            nc.sync.dma_start(out=w[:], in_=t.rearrange("k p n -> p k n"))
            w_sb[name] = w
        bqk_sb = const.tile([128, 8], F32, tag="bqk")
        nc.sync.dma_start(out=bqk_sb[:], in_=bqk[:])
        bvb_sb = const.tile([128, 512], F32, tag="bvb")
        nc.sync.dma_start(out=bvb_sb[:], in_=bvb[:])
        bob_sb = const.tile([128, 512], F32, tag="bob")
        nc.sync.dma_start(out=bob_sb[:], in_=bob[:])
        stat_sb = const.tile([128, 384], BF16, tag="statm")
        nc.sync.dma_start(out=stat_sb[:], in_=statm[:])
        ident_sb = const.tile([128, 128], BF16, tag="identd")
        nc.sync.dma_start(out=ident_sb[:], in_=identd[:])

        # scores MM order: consecutive MMs hit distinct PE strip quadrants
        bo_pat = [0, 1, 1, 0, 0, 1, 1, 0]

        for g in range(n_g8):
            tok0 = g * 512

            # ---- load XT chunk (4 k-chunks x (128, 512)) ----
            xt_t = p_xt.tile([128, 4, 512], BF16, tag="xt")
            for k in range(4):
                nc.sync.dma_start(out=xt_t[:, k, :],
                                  in_=xt[k, :, tok0:tok0 + 512])

            # ---- Q/K projections (transposed: d on partitions) ----
            # HW constraint: every matmul must contract over the full 128
            # partitions (mixed row-strip sets with shared col strips fault
            # the PE).  So Q is kept as two ZERO-PADDED variants: qte has
            # even-head rows + zero bottom half, qto zero top + odd-head
            # rows.  The scores MM streams the full 128-row KT slice; the
            # zero rows of the stationary kill the other head's rows.
            qte_t = p_qkv.tile([128, 4, 512], BF16, tag="qte")
            qto_t = p_qkv.tile([128, 4, 512], BF16, tag="qto")
            if g < 3:  # one memset per qkv pool slot (bufs=3); zeros persist
                nc.vector.memset(qte_t[64:128, :, :], 0)
                nc.vector.memset(qto_t[0:64, :, :], 0)
            kt_t = p_qkv.tile([128, 4, 512], BF16, tag="kt")
            for j in range(4):
                ps = ps_proj.tile([128, 512], F32, tag="proj")
                for k in range(4):
                    nc.tensor.matmul(
                        ps[:], w_sb["wqt"][:, k, 128 * j:128 * (j + 1)],
                        xt_t[:, k, :], start=(k == 0), stop=(k == 3))
                nc.scalar.activation(
                    qte_t[0:64, j, :], ps[0:64, :], AF.Identity,
                    bias=bqk_sb[0:64, j:j + 1], scale=1.0)
                nc.scalar.activation(
                    qto_t[64:128, j, :], ps[64:128, :], AF.Identity,
                    bias=bqk_sb[64:128, j:j + 1], scale=1.0)
            for j in range(4):
                ps = ps_proj.tile([128, 512], F32, tag="proj")
                for k in range(4):
                    nc.tensor.matmul(
                        ps[:], w_sb["wkt"][:, k, 128 * j:128 * (j + 1)],
                        xt_t[:, k, :], start=(k == 0), stop=(k == 3))
                nc.scalar.activation(
                    kt_t[:, j, :], ps[:], AF.Identity,
                    bias=bqk_sb[:, 4 + j:4 + j + 1], scale=1.0)

            # ---- V projection (natural: tokens on partitions) ----
            # Four zero-padded variants for the attn@v stationary, indexed
            # by (attn-row half to match, board): vv[hp][b01].
            v_t = p_qkv.tile([128, 4, 512], BF16, tag="v")
            vv = [[p_qkv.tile([128, 4, 512], BF16, tag=f"vv{hp}{b01}",
                              name=f"vv{hp}{b01}")
                   for b01 in range(2)] for hp in range(2)]
            if g < 3:
                for hp in range(2):
                    for b01 in range(2):
                        nc.vector.memset(
                            vv[hp][b01][64 * (1 - hp):64 * (1 - hp) + 64, :, :], 0)
            for mt in range(4):
                ps = ps_proj.tile([128, 512], F32, tag="proj")
                for k in range(4):
                    nc.tensor.matmul(
                        ps[:], xt_t[:, k, 128 * mt:128 * (mt + 1)],
                        w_sb["wvt"][:, k, :], start=(k == 0), stop=(k == 3))
                nc.vector.tensor_add(v_t[:, mt, :], ps[:], bvb_sb[:])
                # scatter halves into the zero-padded variants (DMA moves
                # across partitions; DVE cannot)
                for b01 in range(2):
                    src = v_t[64 * b01:64 * b01 + 64, mt, :]
                    nc.sync.dma_start(out=vv[0][b01][0:64, mt, :], in_=src)
                    nc.sync.dma_start(out=vv[1][b01][64:128, mt, :], in_=src)

            if stages < 2:
                continue
            # ---- attention: scores / mask / exp / den per board-pair ----
            den_t = p_sc.tile([128, 32], F32, tag="den")
            ms_t = p_sc.tile([128, 4, 512], F32, tag="ms")
            e_t = p_sc.tile([128, 4, 512], BF16, tag="e", bufs=3)
            for p in range(4):   # board pair p: boards (2p, 2p+1) of this g8
                sc = ps_sc.tile([128, 512], F32, tag="sc")
                # PSUM accumulation-group state is per partition: one group
                # per partition half (board parity).
                nh = [0, 0]
                mms = {0: [], 1: []}
                for rep in range(2):
                    for h in range(8):
                        bpar = bo_pat[h] ^ rep
                        bi8 = 2 * p + bpar
                        j, hp = h // 2, h % 2
                        qv = qte_t if hp == 0 else qto_t
                        # skip_group_check: the sim's global group tracker
                        # mis-addresses partition-offset MMs; pending-zero
                        # (per-tensor, drives the data) handles them fine.
                        mms[bpar].append(nc.tensor.matmul(
                            sc[64 * bpar:64 * bpar + 64, 64 * h:64 * h + 64],
                            qv[:, j, 64 * bi8:64 * bi8 + 64],
                            kt_t[:, j, 64 * bi8:64 * bi8 + 64],
                            start=(nh[bpar] == 0), stop=(nh[bpar] == 7),
                            skip_group_check=True))
                        nh[bpar] += 1
                chain(mms[0])
                chain(mms[1])
                nc.vector.tensor_add(ms_t[:, p, 0:384], sc[:, 0:384], stat_sb[:])
                dyn = p_sc.tile([128, 128], BF16, tag="dyn")
                nc.sync.dma_start(out=dyn[:], in_=dynm[g * 4 + p, :, :])
                nc.vector.tensor_add(ms_t[:, p, 384:512], sc[:, 384:512], dyn[:])
                nc.scalar.activation(e_t[:, p, :], ms_t[:, p, :], AF.Exp,
                                     scale=SCALE)
                nc.vector.tensor_reduce(
                    den_t[:, 8 * p:8 * p + 8],
                    e_t[:, p, :].rearrange("p (h t) -> p h t", t=64),
                    axis=mybir.AxisListType.X, op=ALU.add)

            if stages < 2.5:
                continue

            if stages < 3:
                continue
            rden_t = p_sc.tile([128, 32], F32, tag="rden")
            nc.vector.reciprocal(rden_t[:], den_t[:])

            # ---- normalize + PE transpose (128x128 blocks) ----
            at_sb = []
            for r in range(2):   # two pairs per AT bank
                at_ps = ps_at.tile([128, 1024], BF16, tag="at")
                mms = []
                for pp in range(2):
                    p = 2 * r + pp
                    a_t = p_att.tile([128, 512], BF16, tag="a")
                    nc.vector.tensor_mul(
                        a_t[:].rearrange("p (h t) -> p h t", t=64),
                        e_t[:, p, :].rearrange("p (h t) -> p h t", t=64),
                        rden_t[:, 8 * p:8 * p + 8]
                        .rearrange("p (h o) -> p h o", o=1)
                        .broadcast_to((128, 8, 64)))
                    for q in range(4):
                        mms.append(nc.tensor.matmul(
                            at_ps[:, 512 * pp + 128 * q:
                                  512 * pp + 128 * (q + 1)],
                            a_t[:, 128 * q:128 * (q + 1)], ident_sb[:],
                            is_transpose=True,
                            start=(pp == 0 and q == 0),
                            stop=(pp == 1 and q == 3)))
                chain(mms)
                at_t = p_att.tile([128, 1024], BF16, tag="at_sb", bufs=3)
                nc.scalar.copy(at_t[:], at_ps[:])
                at_sb.append(at_t)

            if stages < 4:
                continue
            # ---- attn @ v -> attnout transposed (d on partitions) ----
            ao_sb = []
            for j in range(4):
                ao_ps = ps_ao.tile([128, 512], F32, tag="ao")
                nh = [0, 0]
                mms = {0: [], 1: []}
                for p in range(4):
                    for hp in range(2):
                        h = 2 * j + hp
                        for bpar in range(2):
                            bi8 = 2 * p + bpar
                            rhs = at_sb[p // 2][
                                :, 512 * (p % 2) + 128 * j + 64 * bpar:
                                512 * (p % 2) + 128 * j + 64 * bpar + 64]
                            mms[hp].append(nc.tensor.matmul(
                                ao_ps[64 * hp:64 * hp + 64,
                                      64 * bi8:64 * bi8 + 64],
                                vv[hp][bpar][:, p, 64 * h:64 * h + 64],
                                rhs, start=(nh[hp] == 0), stop=(nh[hp] == 7),
                                skip_group_check=True))
                            nh[hp] += 1
                chain(mms[0])
                chain(mms[1])
                ao_t = p_att.tile([128, 512], BF16, tag="ao_sb", bufs=5)
                nc.vector.tensor_copy(ao_t[:], ao_ps[:])
                ao_sb.append(ao_t)

            if stages < 5:
                continue
            # ---- output projection: Y natural (tokens on partitions) ----
            for mt in range(4):
                ps = ps_proj.tile([128, 512], F32, tag="proj")
                for j in range(4):
                    nc.tensor.matmul(
                        ps[:], ao_sb[j][:, 128 * mt:128 * (mt + 1)],
                        w_sb["wot"][:, j, :], start=(j == 0), stop=(j == 3))
                y_t = p_out.tile([128, 512], F32, tag="y")
                nc.vector.tensor_add(y_t[:], ps[:], bob_sb[:])
                nc.sync.dma_start(
                    out=out[tok0 + 128 * mt:tok0 + 128 * (mt + 1), :],
                    in_=y_t[:])

    nc.finalize()
    return nc


def prep_inputs(x, ray_mask, attack_mask, Wq, bq, Wk, bk, Wv, bv, Wo, bo,
                n_boards=BPC, core=None):
    """Host-side prep: slice per core, transpose x, build masks/bias tiles."""
    TOK = n_boards * S
    eye = np.eye(64, dtype=bool)

    # shared (replicated) tensors
    def wt(W):  # (512,512) -> (4,128,512) bf16, W.T chunked over k
        return np.ascontiguousarray(
            W.T.astype(bf16).reshape(4, 128, 512))

    bqk_h = np.concatenate(
        [bq.astype(np.float32).reshape(4, 128).T,
         bk.astype(np.float32).reshape(4, 128).T], axis=1)  # (128, 8)
    bvb_h = np.broadcast_to(bv.astype(np.float32), (128, 512)).copy()
    bob_h = np.broadcast_to(bo.astype(np.float32), (128, 512)).copy()
    stat = _static_masks()  # (6,64,64) bool
    statm_h = np.where(stat, 0.0, MASKNEG).astype(bf16)
    statm_h = np.tile(statm_h.transpose(1, 0, 2).reshape(64, 384), (2, 1))
    statm_h = np.ascontiguousarray(statm_h)  # (128, 384)
    ident_h = np.eye(128, dtype=bf16)
    shared = dict(wqt=wt(Wq), wkt=wt(Wk), wvt=wt(Wv), wot=wt(Wo),
                  bqk=bqk_h, bvb=bvb_h, bob=bob_h, statm=statm_h,
                  identd=ident_h)

    cores = range(NCORES) if core is None else [core]
    in_maps = []
    for c in cores:
        xs = x[c * n_boards:(c + 1) * n_boards].reshape(TOK, 512)
        xt_h = np.ascontiguousarray(xs.T.astype(bf16)).reshape(4, 128, TOK)
        ray = ray_mask[c * n_boards:(c + 1) * n_boards] | eye
        atk = attack_mask[c * n_boards:(c + 1) * n_boards] | eye
        raym = np.where(ray, 0.0, MASKNEG).astype(bf16)
        atkm = np.where(atk, 0.0, MASKNEG).astype(bf16)
        # (nb/2, 128 rows = s of even|odd board, 128 cols = ray|attack)
        dynm_h = np.empty((n_boards // 2, 128, 128), dtype=bf16)
        dynm_h[:, 0:64, 0:64] = raym[0::2]
        dynm_h[:, 0:64, 64:128] = atkm[0::2]
        dynm_h[:, 64:128, 0:64] = raym[1::2]
        dynm_h[:, 64:128, 64:128] = atkm[1::2]
        in_maps.append(dict(xt=xt_h, dynm=dynm_h, **shared))
    return in_maps


_NC_CACHE = {}


def kernel(**inputs):
    n_boards = BPC
    if "nc" not in _NC_CACHE:
        _NC_CACHE["nc"] = build_nc(n_boards)
    nc = _NC_CACHE["nc"]
    in_maps = prep_inputs(**inputs, n_boards=n_boards)
    res = run_bass_kernel_spmd(nc, in_maps, list(range(NCORES)))
    outs = [res.results[c]["out"].reshape(n_boards, S, D)
            for c in range(NCORES)]
    return np.concatenate(outs, axis=0)


if __name__ == "__main__":
    nc = build_nc()
    print("built ok")


# revision 42
# speedup vs baseline: 1.1841x; 1.1841x over previous
"""Trainium2 Bass kernel for chess-structured multi-head attention (8 cores).

Math (per board b of 2048, S=64 squares, D=512, H=8 heads, HD=64):
  q/k/v = x @ W{q,k,v}.T + b{q,k,v}
  scores_h = q_h k_h^T / 8, masked per head (6 static chess relations,
  ray, attack), softmax over targets, out = concat_h(attn_h v_h) @ Wo.T + bo

Sharding: pure data parallel, 256 boards per core; weights replicated.

Per-core layout strategy (all matmul contractions need K on partitions):
  - x is fed pre-transposed from host: XT (512, 16384) bf16.
  - QT/KT computed transposed (d on partitions) directly: lhsT=WqT chunk,
    rhs=XT chunk.  V computed natural (tokens on partitions): lhsT=XT chunk
    (stationary), rhs=WvT chunk.
  - scores computed natural per (board b, head h): (64 s, 64 t) into a PSUM
    bank holding one board-pair (128, 512): partition half = board parity,
    64-col block = head.  16 small MMs share the bank via one start/stop
    accumulation group (disjoint writes).
  - masking is additive pre-scale (-240 -> -30 after *1/8): one static tile
    (heads 0-5) + per-board-pair dynamic tile (ray|attack), DVE adds.
  - exp on ScalarE (bf16 out), denominators via DVE sub-dim reduce, then
    normalize with a free-dim step-0 broadcast multiply.
  - attn transposed on the PE (128x128 identity transposes, bf16), which
    block-swaps board/head parity; consumed against V or a partition-swapped
    copy of V (cheap SBUF->SBUF DMA) so stationary/moving partition bases
    always match.
  - attn@v gives attnout TRANSPOSED (d on partitions) which feeds the output
    projection as the stationary operand: Y natural (tokens on partitions),
    written straight to DRAM.
"""

import os
import sys
from contextlib import ExitStack

import numpy as np
import ml_dtypes

for _p in ("/opt/trn_rl_repo", os.path.expanduser("~/.axon_site/_ro/trn_rl_repo")):
    if os.path.isdir(_p) and _p not in sys.path:
        sys.path.append(_p)

import concourse.bass as bass
import concourse.tile as tile
from concourse import bacc, mybir
from concourse.bass_utils import run_bass_kernel_spmd

BF16 = mybir.dt.bfloat16
F32 = mybir.dt.float32
bf16 = ml_dtypes.bfloat16

B, S, D, H, HD = 2048, 64, 512, 8, 64
NCORES = 8
BPC = B // NCORES          # boards per core
SCALE = float(1.0 / np.sqrt(HD))
MASKNEG = -240.0           # additive mask value pre-scale (-30 after *SCALE)


def _static_masks():
    sq = np.arange(64)
    r = sq // 8
    f = sq % 8
    ri, rj = r[:, None], r[None, :]
    fi, fj = f[:, None], f[None, :]
    dr = np.abs(ri - rj)
    df = np.abs(fi - fj)
    eye = np.eye(64, dtype=bool)
    file_m = fi == fj
    rank_m = ri == rj
    diag_m = (ri - fi) == (rj - fj)
    adiag_m = (ri + fi) == (rj + fj)
    knight_m = (((dr == 2) & (df == 1)) | ((dr == 1) & (df == 2))) | eye
    king_m = (dr <= 1) & (df <= 1)
    return np.stack([file_m, rank_m, diag_m, adiag_m, knight_m, king_m])


def build_nc(n_boards=BPC, stages=99):
    """Build the single-core Bass program (SPMD across 8 cores)."""
    assert n_boards % 8 == 0
    n_g8 = n_boards // 8          # groups of 8 boards (512 tokens)
    TOK = n_boards * S

    # Bacc (not plain Bass): its compile() pass splits multi-sem waits into
    # EventSemaphore instructions — walrus allows 1 wait per instruction.
    nc = bacc.Bacc(None)

    xt = nc.dram_tensor("xt", [4, 128, TOK], BF16, kind="ExternalInput")
    dynm = nc.dram_tensor("dynm", [n_boards // 2, 128, 128], BF16,
                          kind="ExternalInput")
    wqt = nc.dram_tensor("wqt", [4, 128, 512], BF16, kind="ExternalInput")
    wkt = nc.dram_tensor("wkt", [4, 128, 512], BF16, kind="ExternalInput")
    wvt = nc.dram_tensor("wvt", [4, 128, 512], BF16, kind="ExternalInput")
    wot = nc.dram_tensor("wot", [4, 128, 512], BF16, kind="ExternalInput")
    bqk = nc.dram_tensor("bqk", [128, 8], F32, kind="ExternalInput")
    bvb = nc.dram_tensor("bvb", [128, 512], F32, kind="ExternalInput")
    bob = nc.dram_tensor("bob", [128, 512], F32, kind="ExternalInput")
    statm = nc.dram_tensor("statm", [128, 384], BF16, kind="ExternalInput")
    identd = nc.dram_tensor("identd", [128, 128], BF16, kind="ExternalInput")
    out = nc.dram_tensor("out", [TOK, 512], F32, kind="ExternalOutput")

    AF = mybir.ActivationFunctionType
    ALU = mybir.AluOpType

    def chain(insts):
        # Order-only deps: the start=True MM must execute first and the
        # stop=True MM last; members in between may reorder freely.
        for b in insts[1:]:
            tile.add_dep_helper(b.ins, insts[0].ins, sync=False,
                                reason="psum group start-first")
        for a in insts[1:-1]:
            tile.add_dep_helper(insts[-1].ins, a.ins, sync=False,
                                reason="psum group stop-last")

    with tile.TileContext(nc) as tc, ExitStack() as ctx:
        const = ctx.enter_context(tc.tile_pool(name="const", bufs=1))
        # SBUF working pools
        p_xt = ctx.enter_context(tc.tile_pool(name="xt", bufs=3))
        p_qkv = ctx.enter_context(tc.tile_pool(name="qkv", bufs=3))
        p_att = ctx.enter_context(tc.tile_pool(name="att", bufs=3))
        p_sc = ctx.enter_context(tc.tile_pool(name="sc", bufs=3))
        p_out = ctx.enter_context(tc.tile_pool(name="out", bufs=3))
        # PSUM pools (8 banks total; every tile here is exactly one bank)
        ps_proj = ctx.enter_context(
            tc.tile_pool(name="ps_proj", bufs=3, space="PSUM"))
        ps_sc = ctx.enter_context(
            tc.tile_pool(name="ps_sc", bufs=2, space="PSUM"))
        ps_at = ctx.enter_context(
            tc.tile_pool(name="ps_at", bufs=1, space="PSUM"))
        ps_ao = ctx.enter_context(
            tc.tile_pool(name="ps_ao", bufs=2, space="PSUM"))

        # ---- constants ----
        w_sb = {}
        for name, t in (("wqt", wqt), ("wkt", wkt), ("wvt", wvt), ("wot", wot)):
            w = const.tile([128, 4, 512], BF16, tag=name)
            nc.sync.dma_start(out=w[:], in_=t.rearrange("k p n -> p k n"))
            w_sb[name] = w
        bqk_sb = const.tile([128, 8], F32, tag="bqk")
        nc.sync.dma_start(out=bqk_sb[:], in_=bqk[:])
        bvb_sb = const.tile([128, 512], F32, tag="bvb")
        nc.sync.dma_start(out=bvb_sb[:], in_=bvb[:])
        bob_sb = const.tile([128, 512], F32, tag="bob")
        nc.sync.dma_start(out=bob_sb[:], in_=bob[:])
        stat_sb = const.tile([128, 384], BF16, tag="statm")
        nc.sync.dma_start(out=stat_sb[:], in_=statm[:])
        ident_sb = const.tile([128, 128], BF16, tag="identd")
        nc.sync.dma_start(out=ident_sb[:], in_=identd[:])

        # scores MM order: consecutive MMs hit distinct PE strip quadrants
        bo_pat = [0, 1, 1, 0, 0, 1, 1, 0]

        for g in range(n_g8):
            tok0 = g * 512

            # ---- load XT chunk (4 k-chunks x (128, 512)) ----
            xt_t = p_xt.tile([128, 4, 512], BF16, tag="xt")
            for k in range(4):
                nc.sync.dma_start(out=xt_t[:, k, :],
                                  in_=xt[k, :, tok0:tok0 + 512])

            # ---- Q/K projections (transposed: d on partitions) ----
            # HW constraint: every matmul must contract over the full 128
            # partitions (mixed row-strip sets with shared col strips fault
            # the PE).  So Q is kept as two ZERO-PADDED variants: qte has
            # even-head rows + zero bottom half, qto zero top + odd-head
            # rows.  The scores MM streams the full 128-row KT slice; the
            # zero rows of the stationary kill the other head's rows.
            qte_t = p_qkv.tile([128, 4, 512], BF16, tag="qte")
            qto_t = p_qkv.tile([128, 4, 512], BF16, tag="qto")
            if g < 3:  # one memset per qkv pool slot (bufs=3); zeros persist
                nc.vector.memset(qte_t[64:128, :, :], 0)
                nc.vector.memset(qto_t[0:64, :, :], 0)
            kt_t = p_qkv.tile([128, 4, 512], BF16, tag="kt")
            for j in range(4):
                ps = ps_proj.tile([128, 512], F32, tag="proj")
                for k in range(4):
                    nc.tensor.matmul(
                        ps[:], w_sb["wqt"][:, k, 128 * j:128 * (j + 1)],
                        xt_t[:, k, :], start=(k == 0), stop=(k == 3))
                nc.scalar.activation(
                    qte_t[0:64, j, :], ps[0:64, :], AF.Identity,
                    bias=bqk_sb[0:64, j:j + 1], scale=1.0)
                nc.scalar.activation(
                    qto_t[64:128, j, :], ps[64:128, :], AF.Identity,
                    bias=bqk_sb[64:128, j:j + 1], scale=1.0)
            for j in range(4):
                ps = ps_proj.tile([128, 512], F32, tag="proj")
                for k in range(4):
                    nc.tensor.matmul(
                        ps[:], w_sb["wkt"][:, k, 128 * j:128 * (j + 1)],
                        xt_t[:, k, :], start=(k == 0), stop=(k == 3))
                nc.scalar.activation(
                    kt_t[:, j, :], ps[:], AF.Identity,
                    bias=bqk_sb[:, 4 + j:4 + j + 1], scale=1.0)

            # ---- V projection (natural: tokens on partitions) ----
            # Four zero-padded variants for the attn@v stationary, indexed
            # by (attn-row half to match, board): vv[hp][b01].
            v_t = p_qkv.tile([128, 4, 512], BF16, tag="v")
            vv = [[p_qkv.tile([128, 4, 512], BF16, tag=f"vv{hp}{b01}",
                              name=f"vv{hp}{b01}")
                   for b01 in range(2)] for hp in range(2)]
            if g < 3:
                for hp in range(2):
                    for b01 in range(2):
                        nc.vector.memset(
                            vv[hp][b01][64 * (1 - hp):64 * (1 - hp) + 64, :, :], 0)
            for mt in range(4):
                ps = ps_proj.tile([128, 512], F32, tag="proj")
                for k in range(4):
                    nc.tensor.matmul(
                        ps[:], xt_t[:, k, 128 * mt:128 * (mt + 1)],
                        w_sb["wvt"][:, k, :], start=(k == 0), stop=(k == 3))
                nc.vector.tensor_add(v_t[:, mt, :], ps[:], bvb_sb[:])
                # scatter halves into the zero-padded variants (DMA moves
                # across partitions; DVE cannot)
                for b01 in range(2):
                    src = v_t[64 * b01:64 * b01 + 64, mt, :]
                    nc.sync.dma_start(out=vv[0][b01][0:64, mt, :], in_=src)
                    nc.sync.dma_start(out=vv[1][b01][64:128, mt, :], in_=src)

            if stages < 2:
                continue
            # ---- attention: scores / mask / exp / den per board-pair ----
            den_t = p_sc.tile([128, 32], F32, tag="den")
            ms_t = p_sc.tile([128, 4, 512], F32, tag="ms")
            e_t = p_sc.tile([128, 4, 512], BF16, tag="e", bufs=4)
            dyn_t = p_sc.tile([128, 4, 128], BF16, tag="dyn")
            nc.sync.dma_start(out=dyn_t[:], in_=dynm[4 * g:4 * g + 4, :, :]
                              .rearrange("q p c -> p q c"))
            for p in range(4):   # board pair p: boards (2p, 2p+1) of this g8
                sc = ps_sc.tile([128, 512], F32, tag="sc")
                # PSUM accumulation-group state is per partition: one group
                # per partition half (board parity).
                nh = [0, 0]
                mms = {0: [], 1: []}
                for rep in range(2):
                    for h in range(8):
                        bpar = bo_pat[h] ^ rep
                        bi8 = 2 * p + bpar
                        j, hp = h // 2, h % 2
                        qv = qte_t if hp == 0 else qto_t
                        # skip_group_check: the sim's global group tracker
                        # mis-addresses partition-offset MMs; pending-zero
                        # (per-tensor, drives the data) handles them fine.
                        mms[bpar].append(nc.tensor.matmul(
                            sc[64 * bpar:64 * bpar + 64, 64 * h:64 * h + 64],
                            qv[:, j, 64 * bi8:64 * bi8 + 64],
                            kt_t[:, j, 64 * bi8:64 * bi8 + 64],
                            start=(nh[bpar] == 0), stop=(nh[bpar] == 7),
                            skip_group_check=True))
                        nh[bpar] += 1
                chain(mms[0])
                chain(mms[1])
                nc.vector.tensor_add(ms_t[:, p, 0:384], sc[:, 0:384], stat_sb[:])
                nc.vector.tensor_add(ms_t[:, p, 384:512], sc[:, 384:512],
                                     dyn_t[:, p, :])
                nc.scalar.activation(e_t[:, p, :], ms_t[:, p, :], AF.Exp,
                                     scale=SCALE)
                nc.vector.tensor_reduce(
                    den_t[:, 8 * p:8 * p + 8],
                    e_t[:, p, :].rearrange("p (h t) -> p h t", t=64),
                    axis=mybir.AxisListType.X, op=ALU.add)

            if stages < 2.5:
                continue

            if stages < 3:
                continue
            rden_t = p_sc.tile([128, 32], F32, tag="rden")
            nc.vector.reciprocal(rden_t[:], den_t[:])

            # ---- normalize + PE transpose (128x128 blocks) ----
            at_sb = []
            for r in range(2):   # two pairs per AT bank
                at_ps = ps_at.tile([128, 1024], BF16, tag="at")
                mms = []
                for pp in range(2):
                    p = 2 * r + pp
                    a_t = p_att.tile([128, 512], BF16, tag="a")
                    nc.vector.tensor_mul(
                        a_t[:].rearrange("p (h t) -> p h t", t=64),
                        e_t[:, p, :].rearrange("p (h t) -> p h t", t=64),
                        rden_t[:, 8 * p:8 * p + 8]
                        .rearrange("p (h o) -> p h o", o=1)
                        .broadcast_to((128, 8, 64)))
                    for q in range(4):
                        mms.append(nc.tensor.matmul(
                            at_ps[:, 512 * pp + 128 * q:
                                  512 * pp + 128 * (q + 1)],
                            a_t[:, 128 * q:128 * (q + 1)], ident_sb[:],
                            is_transpose=True,
                            start=(pp == 0 and q == 0),
                            stop=(pp == 1 and q == 3)))
                chain(mms)
                at_t = p_att.tile([128, 1024], BF16, tag="at_sb", bufs=3)
                nc.scalar.copy(at_t[:], at_ps[:])
                at_sb.append(at_t)

            if stages < 4:
                continue
            # ---- attn @ v -> attnout transposed (d on partitions) ----
            ao_sb = []
            for j in range(4):
                ao_ps = ps_ao.tile([128, 512], F32, tag="ao")
                nh = [0, 0]
                mms = {0: [], 1: []}
                for p in range(4):
                    for hp in range(2):
                        h = 2 * j + hp
                        for bpar in range(2):
                            bi8 = 2 * p + bpar
                            rhs = at_sb[p // 2][
                                :, 512 * (p % 2) + 128 * j + 64 * bpar:
                                512 * (p % 2) + 128 * j + 64 * bpar + 64]
                            mms[hp].append(nc.tensor.matmul(
                                ao_ps[64 * hp:64 * hp + 64,
                                      64 * bi8:64 * bi8 + 64],
                                vv[hp][bpar][:, p, 64 * h:64 * h + 64],
                                rhs, start=(nh[hp] == 0), stop=(nh[hp] == 7),
                                skip_group_check=True))
                            nh[hp] += 1
                chain(mms[0])
                chain(mms[1])
                ao_t = p_att.tile([128, 512], BF16, tag="ao_sb", bufs=5)
                nc.vector.tensor_copy(ao_t[:], ao_ps[:])
                ao_sb.append(ao_t)

            if stages < 5:
                continue
            # ---- output projection: Y natural (tokens on partitions) ----
            for mt in range(4):
                ps = ps_proj.tile([128, 512], F32, tag="proj")
                for j in range(4):
                    nc.tensor.matmul(
                        ps[:], ao_sb[j][:, 128 * mt:128 * (mt + 1)],
                        w_sb["wot"][:, j, :], start=(j == 0), stop=(j == 3))
                y_t = p_out.tile([128, 512], F32, tag="y")
                nc.vector.tensor_add(y_t[:], ps[:], bob_sb[:])
                nc.sync.dma_start(
                    out=out[tok0 + 128 * mt:tok0 + 128 * (mt + 1), :],
                    in_=y_t[:])

    nc.finalize()
    return nc


def prep_inputs(x, ray_mask, attack_mask, Wq, bq, Wk, bk, Wv, bv, Wo, bo,
                n_boards=BPC, core=None):
    """Host-side prep: slice per core, transpose x, build masks/bias tiles."""
    TOK = n_boards * S
    eye = np.eye(64, dtype=bool)

    # shared (replicated) tensors
    def wt(W):  # (512,512) -> (4,128,512) bf16, W.T chunked over k
        return np.ascontiguousarray(
            W.T.astype(bf16).reshape(4, 128, 512))

    bqk_h = np.concatenate(
        [bq.astype(np.float32).reshape(4, 128).T,
         bk.astype(np.float32).reshape(4, 128).T], axis=1)  # (128, 8)
    bvb_h = np.broadcast_to(bv.astype(np.float32), (128, 512)).copy()
    bob_h = np.broadcast_to(bo.astype(np.float32), (128, 512)).copy()
    stat = _static_masks()  # (6,64,64) bool
    statm_h = np.where(stat, 0.0, MASKNEG).astype(bf16)
    statm_h = np.tile(statm_h.transpose(1, 0, 2).reshape(64, 384), (2, 1))
    statm_h = np.ascontiguousarray(statm_h)  # (128, 384)
    ident_h = np.eye(128, dtype=bf16)
    shared = dict(wqt=wt(Wq), wkt=wt(Wk), wvt=wt(Wv), wot=wt(Wo),
                  bqk=bqk_h, bvb=bvb_h, bob=bob_h, statm=statm_h,
                  identd=ident_h)

    cores = range(NCORES) if core is None else [core]
    in_maps = []
    for c in cores:
        xs = x[c * n_boards:(c + 1) * n_boards].reshape(TOK, 512)
        xt_h = np.ascontiguousarray(xs.T.astype(bf16)).reshape(4, 128, TOK)
        ray = ray_mask[c * n_boards:(c + 1) * n_boards] | eye
        atk = attack_mask[c * n_boards:(c + 1) * n_boards] | eye
        raym = np.where(ray, 0.0, MASKNEG).astype(bf16)
        atkm = np.where(atk, 0.0, MASKNEG).astype(bf16)
        # (nb/2, 128 rows = s of even|odd board, 128 cols = ray|attack)
        dynm_h = np.empty((n_boards // 2, 128, 128), dtype=bf16)
        dynm_h[:, 0:64, 0:64] = raym[0::2]
        dynm_h[:, 0:64, 64:128] = atkm[0::2]
        dynm_h[:, 64:128, 0:64] = raym[1::2]
        dynm_h[:, 64:128, 64:128] = atkm[1::2]
        in_maps.append(dict(xt=xt_h, dynm=dynm_h, **shared))
    return in_maps


_NC_CACHE = {}


def kernel(**inputs):
    n_boards = BPC
    if "nc" not in _NC_CACHE:
        _NC_CACHE["nc"] = build_nc(n_boards)
    nc = _NC_CACHE["nc"]
    in_maps = prep_inputs(**inputs, n_boards=n_boards)
    res = run_bass_kernel_spmd(nc, in_maps, list(range(NCORES)))
    outs = [res.results[c]["out"].reshape(n_boards, S, D)
            for c in range(NCORES)]
    return np.concatenate(outs, axis=0)


if __name__ == "__main__":
    nc = build_nc()
    print("built ok")
